# revision 29
# baseline (speedup 1.0000x reference)
"""Trainium2 Bass kernel for nn_Attention_51634096833229.

Conv-projection attention block (CvT-style): depthwise 3x3 conv + BN on the
28x28 token image for each of q/k/v, linear qkv projections, 3-head attention
over 785 tokens (784 image + 1 cls), output projection.

Sharding: data-parallel over batch, B=32 -> 4 samples per core on 8 cores.

Design notes (all fp16 on-chip, ~1.5x the fp32 baseline in TimelineSim):
  - scalar_tensor_tensor has NO DVE fast modes (1.04ns/elem); only 2-scalar
    tensor_scalar (4x w/ fp16+SBUF) and tensor_tensor (2x w/ fp16) do. The
    conv is split: PE_CONV groups run as 9 diagonal-matmul taps accumulated
    in PSUM (row-split into two 1-bank halves) + an ACT Identity bias-add
    copy-out; the (q,ch1)+(k,ch1) groups are STACKED into one 128-partition
    DVE group (image rows duplicated into both partition halves, stacked
    per-partition weights) of 9 tensor_scalar products + a 4-op pane-tree of
    tensor_tensor adds; (v,ch1) runs the same solo.
  - x is loaded in 7 blocks of 112 tokens (= 4 image rows), PE-transposed and
    copied straight into the zero-padded fp16 [c,30,30] image.
  - software pipeline emits stage A(b) [load/conv/projections], attention
    T(b-1) [scores/exp/PV, PE+ACT only] and normalize N(b-2) so in-order
    engine streams overlap samples; PSUM pools are split per stage (scores
    2x2-bank, A-stage matmuls 2x1-bank, pv 1x2-bank) to avoid cross-stage
    ring coupling.
  - normalize copies pv out of PSUM immediately (freeing the single pv
    buffer), then reciprocal + Pool partition-broadcast + fp16 multiply.
  - GPSIMD/Pool cannot touch PSUM on HW: it only gets SBUF->SBUF work
    (v-split, pad duplication, broadcasts, some conv products).
  - ACT runs Exp, Copy and Identity - all in the same activation-table
    function set, so no table reloads.
"""

import sys

sys.path.insert(0, "/opt/trn_rl_repo")

import numpy as np

import concourse.bass as bass
import concourse.mybir as mybir
import concourse.tile as tile
from concourse import bacc
from concourse.masks import make_identity
from concourse.bass_utils import run_bass_kernel_spmd

F32 = mybir.dt.float32
F16 = mybir.dt.float16
AF = mybir.ActivationFunctionType
OP = mybir.AluOpType

B, T, C, CO, NH, D = 32, 785, 192, 192, 3, 64
HH = WW = 28
NCORES = 8
BPC = B // NCORES  # samples per core
SCALE = float(CO) ** -0.5
BN_EPS = 1e-5

# token blocks of 128 along T (attention tiling)
TBLK = [(i * 128, min(128, T - i * 128)) for i in range((T + 127) // 128)]
# image-token blocks of 112 = 4 rows of 28 (transpose/load tiling)
NXB = 7
# channel chunks along C=192
CCH = [(0, 128), (128, 64)]
# N segments within 785 (psum bank = 512 f32)
NSEG = [(0, 512), (512, T - 512)]

# conv groups (proj, chunk) computed on PE via diagonal matmuls
PE_CONV = {(0, 0), (1, 0), (2, 0)}
# engine assignment: "v" = DVE, "g" = Pool/gpsimd (SBUF-only!), "a" = ACT.
# GPSIMD cannot access PSUM, so PSUM-touching ops must be "v" or "a".
ENG = {
    "padcopy0": "v", "padcopy1": "v",
    "clscopy": "g",
    "qkcopy0": "a", "qkcopy1": "a",
    "vstage": "v", "vsplit": "g",
    "pvcopy": "v", "rcopy": "v", "ttdiv": "v",
    "pebias": "a",
    "outcopy": "v",
}
# taps of DVE conv groups whose products run on Pool (SBUF->SBUF)
POOL_TAPS = {0, 1}
PSUM_BUFS = [2, 2, 1]
PRODB = 1
EBUFS = 4
NORM_AFTER_A = True


def _pad3(pad_ap):
    return pad_ap.rearrange("p (y x) -> p y x", y=30, x=30)


def _conv_shift_ap(pad_ap, dy, dx):
    """3D AP view [P, 28, 28] of the padded [P, 30*30] image for tap (dy,dx)."""
    return _pad3(pad_ap)[:, dy:dy + 28, dx:dx + 28]


def _img3(ap):
    """[P, 784] -> [P, 28, 28] view."""
    return ap.rearrange("p (y x) -> p y x", y=28, x=28)


def build_bass():
    nc = bacc.Bacc(None)
    x_d = nc.declare_dram_parameter("x", [BPC, T, C], F32, isOutput=False)
    wqkvT_d = nc.declare_dram_parameter("wqkvT", [3, C, CO], F16, isOutput=False)
    wconv_d = nc.declare_dram_parameter("wconv", [C, 27], F32, isOutput=False)
    bnt_d = nc.declare_dram_parameter("bnt", [C, 3], F32, isOutput=False)
    wdg0_d = nc.declare_dram_parameter("wdg0", [27 * 128, 128], F16, isOutput=False)
    wdg1_d = nc.declare_dram_parameter("wdg1", [27 * 64, 64], F16, isOutput=False)
    wcs_d = nc.declare_dram_parameter("wcs", [128, 10], F32, isOutput=False)
    wpa_d = nc.declare_dram_parameter("wpa", [C + 1, CO], F16, isOutput=False)
    out_d = nc.declare_dram_parameter("out", [BPC, T, CO], F32, isOutput=True)

    def eng(key):
        return {"v": nc.vector, "g": nc.gpsimd, "a": None}[ENG[key]]

    def copy_via(key, dst, src):
        e = ENG[key]
        if e == "a":
            nc.scalar.activation(dst, src, AF.Copy)
        else:
            {"v": nc.vector, "g": nc.gpsimd}[e].tensor_copy(dst, src)

    from contextlib import ExitStack
    with tile.TileContext(nc) as tc, ExitStack() as es:
        consts = es.enter_context(tc.tile_pool(name="consts", bufs=1))
        psS = es.enter_context(tc.tile_pool(name="psS", bufs=PSUM_BUFS[0], space="PSUM"))
        psM = es.enter_context(tc.tile_pool(name="psM", bufs=PSUM_BUFS[1], space="PSUM"))
        psP = es.enter_context(tc.tile_pool(name="psP", bufs=PSUM_BUFS[2], space="PSUM"))
        xload = es.enter_context(tc.tile_pool(name="xload", bufs=2))
        yp = es.enter_context(tc.tile_pool(name="y", bufs=2))
        prodp = es.enter_context(tc.tile_pool(name="prod", bufs=PRODB))
        q4p = es.enter_context(tc.tile_pool(name="q4", bufs=2))
        vstp = es.enter_context(tc.tile_pool(name="vst", bufs=3))
        qkp = es.enter_context(tc.tile_pool(name="qk", bufs=2))
        ep = es.enter_context(tc.tile_pool(name="E", bufs=EBUFS))
        op_ = es.enter_context(tc.tile_pool(name="osb", bufs=2))
        smallp = es.enter_context(tc.tile_pool(name="small", bufs=2))
        if True:
            ident = consts.tile([128, 128], F32, tag="ident", name="ident")
            make_identity(nc, ident[:])

            # weights into SBUF (batched: one DMA per channel chunk)
            wqA = []
            for ci, (c0, cp) in enumerate(CCH):
                t = consts.tile([cp, 3 * CO], F16, tag=f"wqA{ci}", name=f"wqA{ci}")
                nc.sync.dma_start(
                    t[:].rearrange("p (i o) -> p i o", i=3, o=CO),
                    wqkvT_d[:, c0:c0 + cp, :].rearrange("i c o -> c i o"))
                wqA.append(t)
            wq_sb = [[wqA[ci][:, i * CO:(i + 1) * CO] for ci in range(2)]
                     for i in range(3)]
            wc_sb, bnt_sb = [], []
            for ci, (c0, cp) in enumerate(CCH):
                t = consts.tile([cp, 30], F32, tag=f"wcb{ci}", name=f"wcb{ci}")
                nc.sync.dma_start(t[:, 0:27], wconv_d[c0:c0 + cp, :])
                nc.sync.dma_start(t[:, 27:30], bnt_d[c0:c0 + cp, :])
                wc_sb.append(t[:, 0:27])
                bnt_sb.append(t[:, 27:30])
            # diagonal conv-weight matrices for PE-side conv groups (one DMA each)
            wdA = []
            for ci, (c0, cp) in enumerate(CCH):
                t = consts.tile([cp, 27 * cp], F16, tag=f"wdA{ci}", name=f"wdA{ci}")
                src = wdg0_d if ci == 0 else wdg1_d
                nc.sync.dma_start(
                    t[:].rearrange("p (k j) -> p k j", k=27, j=cp),
                    src[:].rearrange("(k p) j -> p k j", p=cp))
                wdA.append(t)
            wdiag = {}
            for i in range(3):
                for ci, (c0, cp) in enumerate(CCH):
                    for tap in range(9):
                        kk = i * 9 + tap
                        wdiag[(i, ci, tap)] = wdA[ci][:, kk * cp:(kk + 1) * cp]
            wqSt = consts.tile([128, CO], F16, tag="wqSt", name="wqSt")
            nc.sync.dma_start(wqSt[0:64, :], wqkvT_d[0, 128:192, :])
            nc.sync.dma_start(wqSt[64:128, :], wqkvT_d[1, 128:192, :])
            wcs_sb = consts.tile([128, 10], F32, tag="wcs", name="wcs")
            nc.sync.dma_start(wcs_sb[:], wcs_d[:, :])
            wpa0 = consts.tile([128, CO], F16, tag="wpa0", name="wpa0")
            nc.sync.dma_start(wpa0[:], wpa_d[0:128, :])
            wpa1 = consts.tile([65, CO], F16, tag="wpa1", name="wpa1")
            nc.sync.dma_start(wpa1[:], wpa_d[128:193, :])

            # parity-persistent tiles
            pads = [[consts.tile([128, 900], F16, tag=f"pad{p}{ci}",
                                 name=f"pad{p}{ci}")
                     for ci in range(2)] for p in range(2)]
            for p in range(2):
                for ci in range(2):
                    nc.vector.memset(pads[p][ci][:], 0.0)
            vaug = [[[consts.tile([128, 65], F16, tag=f"va{p}{h}{tb}",
                                  name=f"va{p}{h}{tb}")
                      for tb in range(len(TBLK))] for h in range(NH)]
                    for p in range(BPC)]
            for p in range(BPC):
                for h in range(NH):
                    for tb, (t0, tn) in enumerate(TBLK):
                        nc.vector.memset(vaug[p][h][tb][:, 64:65], 1.0)
            aT0 = [consts.tile([128, T], F16, tag=f"aT0{p}", name=f"aT0{p}")
                   for p in range(BPC)]
            aT1 = [consts.tile([65, T], F16, tag=f"aT1{p}", name=f"aT1{p}")
                   for p in range(BPC)]
            for p in range(BPC):
                nc.vector.memset(aT1[p][64:65, :], 1.0)
            cls2 = [consts.tile([128, 2], F32, tag=f"cls{p}", name=f"cls{p}")
                    for p in range(BPC)]

            def head_rows(qk, h):
                """[64, T] slice of qT/kT chunks for head h."""
                if h < 2:
                    return qk[0][h * 64:(h + 1) * 64, :]
                return qk[1][0:64, :]

            qkT_of = {}

            def emit_A(b):
                """Load, transpose->pads, conv->ys, qk/v projections."""
                par = b % 2
                pad, va = pads[par], vaug[b]
                xin = xload.tile([112, NXB * C], F32, tag="xin", name="xin")
                nc.sync.dma_start(
                    xin[:].rearrange("p (n c) -> p n c", n=NXB, c=C),
                    x_d[b, 1:785, :].rearrange("(n p) c -> p n c", p=112))
                nc.sync.dma_start(cls2[b][:, 0:1], x_d[b, 0, 0:128])
                nc.sync.dma_start(cls2[b][0:64, 1:2], x_d[b, 0, 128:192])

                # ---- PE transpose straight into the padded image (fp16) ----
                for i in range(NXB):
                    ps = psM.tile([128, 512], F32, tag="mm", name="mm")
                    xl = xin[:, i * C:(i + 1) * C]
                    nc.tensor.transpose(ps[0:128, 0:112], xl[:, 0:128],
                                        ident[0:112, 0:112])
                    nc.tensor.transpose(ps[0:64, 112:224], xl[:, 128:192],
                                        ident[0:112, 0:112])
                    for ci, (c0, cp) in enumerate(CCH):
                        src = ps[0:cp, 112 * ci:112 * ci + 112]
                        dst3 = _pad3(pad[ci][:])[:, 1 + 4 * i:5 + 4 * i, 1:29]
                        copy_via("padcopy0" if (i + ci) % 2 == 0 else "padcopy1",
                                 dst3[0:cp],
                                 src.rearrange("p (a x) -> p a x", a=4, x=28))
                        if ci == 1:
                            # duplicate chunk1 rows into partitions 64:128 for
                            # the stacked q|k conv group (SBUF->SBUF on Pool)
                            nc.gpsimd.tensor_copy(
                                _pad3(pad[1][64:128, :])[
                                    :, 1 + 4 * i:5 + 4 * i, 1:29],
                                dst3[0:64])

                # ---- depthwise conv + BN -> y (fp16), cls col prepended ----
                ys = [[None, None] for _ in range(3)]
                # stacked (q,ch1)|(k,ch1) group: one 128-partition DVE pass
                ySt = yp.tile([128, T], F16, tag="ySt", name="ySt")
                ySt3 = _img3(ySt[:, 1:T])
                prS = prodp.tile([128, 9 * 784], F16, tag="prS", name="prS")
                pS3 = prS[:].rearrange("p (n f) -> p n f", n=9, f=784)
                for tap in range(9):
                    dy, dx = tap // 3, tap % 3
                    sh = _conv_shift_ap(pad[1][:], dy, dx)
                    wcol = wcs_sb[:, tap:tap + 1]
                    ve = nc.gpsimd if tap in POOL_TAPS else nc.vector
                    if tap == 8:
                        ve.tensor_scalar(
                            pS3[:, 8, :].rearrange("p (a f) -> p a f",
                                                   a=1, f=784),
                            sh, wcol, wcs_sb[:, 9:10], OP.mult, OP.add)
                    else:
                        ve.tensor_scalar(
                            pS3[:, tap, :].rearrange("p (a f) -> p a f",
                                                     a=1, f=784),
                            sh, wcol, None, OP.mult)
                qS = q4p.tile([128, 4 * 784], F16, tag="q4S", name="q4S")
                qS3 = qS[:].rearrange("p (n f) -> p n f", n=4, f=784)
                nc.vector.tensor_tensor(
                    qS3, pS3[:, 0:8:2, :], pS3[:, 1:8:2, :], OP.add)
                nc.vector.tensor_tensor(
                    qS3[:, 0:2, :], qS3[:, 0:2, :], qS3[:, 2:4, :], OP.add)
                nc.vector.tensor_tensor(
                    qS3[:, 0, :], qS3[:, 0, :], qS3[:, 1, :], OP.add)
                nc.vector.tensor_tensor(
                    ySt3.rearrange("p y x -> p (y x)"),
                    qS3[:, 0, :], pS3[:, 8, :], OP.add)
                copy_via("clscopy", ySt[0:64, 0:1], cls2[b][0:64, 1:2])
                copy_via("clscopy", ySt[64:128, 0:1], cls2[b][0:64, 1:2])
                ys[0][1] = ySt[0:64, :]
                ys[1][1] = ySt[64:128, :]

                for i in range(3):
                    for ci, (c0, cp) in enumerate(CCH):
                        if ys[i][ci] is not None:
                            continue
                        y = yp.tile([cp, T], F16, tag=f"y{i}{ci}", name=f"y{i}{ci}")
                        y3 = _img3(y[:, 1:T])
                        if (i, ci) in PE_CONV:
                            # 9 diagonal-matmul taps accumulate in PSUM,
                            # split into two 14-row halves (1 bank each)
                            for r0 in (0, 14):
                                yps = psM.tile([128, 512], F32, tag="mm",
                                               name="mm")
                                for tap in range(9):
                                    dy, dx = tap // 3, tap % 3
                                    sh = _pad3(pad[ci][0:cp, :])[
                                        :, dy + r0:dy + r0 + 14, dx:dx + 28]
                                    nc.tensor.matmul(
                                        yps[0:cp, 0:392],
                                        wdiag[(i, ci, tap)], sh,
                                        start=(tap == 0), stop=(tap == 8))
                                ysrc = yps[0:cp, 0:392].rearrange(
                                    "p (a x) -> p a x", a=14, x=28)
                                if ENG["pebias"] == "a":
                                    nc.scalar.activation(
                                        y3[:, r0:r0 + 14, :], ysrc,
                                        AF.Identity,
                                        bias=bnt_sb[ci][:, i:i + 1])
                                else:
                                    nc.vector.tensor_scalar(
                                        y3[:, r0:r0 + 14, :], ysrc,
                                        bnt_sb[ci][:, i:i + 1], None, OP.add)
                        else:
                            # DVE: 9 fp16 4x products + pane-tree of 2x adds
                            pr = prodp.tile([cp, 9 * 784], F16, tag=f"pr{ci}",
                                            name=f"pr{ci}")
                            p3 = pr[:].rearrange("p (n f) -> p n f", n=9, f=784)
                            for tap in range(9):
                                dy, dx = tap // 3, tap % 3
                                sh = _conv_shift_ap(pad[ci][0:cp, :], dy, dx)
                                wcol = wc_sb[ci][:, i * 9 + tap:i * 9 + tap + 1]
                                ve = (nc.gpsimd if tap in POOL_TAPS
                                      else nc.vector)
                                if tap == 8:
                                    ve.tensor_scalar(
                                        p3[:, 8, :].rearrange("p (a f) -> p a f",
                                                              a=1, f=784),
                                        sh, wcol, bnt_sb[ci][:, i:i + 1],
                                        OP.mult, OP.add)
                                else:
                                    ve.tensor_scalar(
                                        p3[:, tap, :].rearrange(
                                            "p (a f) -> p a f", a=1, f=784),
                                        sh, wcol, None, OP.mult)
                            q4 = q4p.tile([cp, 4 * 784], F16, tag=f"q4{ci}",
                                          name=f"q4{ci}")
                            q43 = q4[:].rearrange("p (n f) -> p n f", n=4, f=784)
                            nc.vector.tensor_tensor(
                                q43,
                                p3[:, 0:8:2, :], p3[:, 1:8:2, :], OP.add)
                            nc.vector.tensor_tensor(
                                q43[:, 0:2, :], q43[:, 0:2, :], q43[:, 2:4, :],
                                OP.add)
                            nc.vector.tensor_tensor(
                                q43[:, 0, :], q43[:, 0, :], q43[:, 1, :], OP.add)
                            nc.vector.tensor_tensor(
                                y3.rearrange("p y x -> p (y x)"),
                                q43[:, 0, :], p3[:, 8, :], OP.add)
                        copy_via("clscopy", y[:, 0:1],
                                 cls2[b][0:cp, ci:ci + 1])
                        ys[i][ci] = y[:, :]

                # ---- q,k feature-major projections -> qT,kT fp16 ----
                qkT = []  # [i][chunk]
                for i in range(2):
                    row = []
                    for ob, (o0, osz) in enumerate(CCH):
                        dst = qkp.tile([osz, T], F16, tag=f"qk{i}{ob}",
                                       name=f"qk{i}{ob}")
                        for si, (n0, nn) in enumerate(NSEG):
                            ps = psM.tile([128, 512], F32, tag="mm", name="mm")
                            for ci in range(2):
                                # ch1 of q/k lives in the stacked ySt tile at
                                # base partition i*64; weights must match base
                                lhsT = (wqSt[i * 64:(i + 1) * 64, o0:o0 + osz]
                                        if ci == 1 else
                                        wq_sb[i][ci][:, o0:o0 + osz])
                                nc.tensor.matmul(
                                    ps[0:osz, 0:nn],
                                    lhsT,
                                    ys[i][ci][:, n0:n0 + nn],
                                    start=(ci == 0), stop=(ci == 1))
                            copy_via("qkcopy0" if (ob + si) % 2 == 0
                                     else "qkcopy1",
                                     dst[:, n0:n0 + nn], ps[0:osz, 0:nn])
                        row.append(dst)
                    qkT.append(row)
                qkT_of[b] = qkT

                # ---- v token-major -> per-head vaug ----
                for tb, (t0, tn) in enumerate(TBLK):
                    ps = psM.tile([128, 512], F32, tag="mm", name="mm")
                    for ci in range(2):
                        nc.tensor.matmul(
                            ps[0:tn, 0:CO],
                            ys[2][ci][:, t0:t0 + tn],
                            wq_sb[2][ci],
                            start=(ci == 0), stop=(ci == 1))
                    vst = vstp.tile([128, CO], F16, tag="vst", name="vst")
                    copy_via("vstage", vst[0:tn, :], ps[0:tn, 0:CO])
                    for h in range(NH):
                        copy_via("vsplit", va[h][tb][0:tn, 0:64],
                                 vst[0:tn, h * 64:(h + 1) * 64])

            pv_of = {}

            def emit_attn(b):
                """Scores, exp, PV accumulate (PE/ACT only)."""
                va, qkT = vaug[b], qkT_of.pop(b)
                for h in range(NH):
                    kh = head_rows(qkT[1], h)
                    qh = head_rows(qkT[0], h)
                    pv = psP.tile([128, T], F32, tag="pv", name="pv")
                    for tb, (t0, tn) in enumerate(TBLK):
                        e = ep.tile([128, T], F16, tag="E", name="E")
                        ss = psS.tile([128, T], F32, tag="ss", name="ss")
                        for (n0, nn) in NSEG:
                            nc.tensor.matmul(
                                ss[0:tn, n0:n0 + nn],
                                kh[:, t0:t0 + tn], qh[:, n0:n0 + nn],
                                start=True, stop=True)
                        nc.scalar.activation(
                            e[0:tn, 0:T], ss[0:tn, 0:T],
                            AF.Exp, scale=SCALE)
                        for (n0, nn) in NSEG:
                            nc.tensor.matmul(
                                pv[0:65, n0:n0 + nn],
                                va[h][tb][0:tn, 0:65],
                                e[0:tn, n0:n0 + nn],
                                start=(tb == 0), stop=(tb == len(TBLK) - 1))
                    pv_of[(b, h)] = pv

            def emit_norm(b):
                """Copy out pv (freeing PSUM fast), then normalize on
                SBUF-only engines: broadcast denominators + divide."""
                for h in range(NH):
                    pv = pv_of.pop((b, h))
                    aTu = smallp.tile([64, T], F16, tag="aTu", name="aTu")
                    copy_via("pvcopy", aTu[:], pv[0:64, 0:T])
                    r = smallp.tile([1, T], F16, tag="r", name="r")
                    with nc.allow_low_precision(
                            reason="softmax denom reciprocal in fp16"):
                        eng("rcopy").reciprocal(r[0:1, :], pv[64:65, 0:T])
                    rb = smallp.tile([64, T], F16, tag="rb", name="rb")
                    nc.gpsimd.partition_broadcast(rb[:], r[0:1, :])
                    dst = (aT0[b][h * 64:(h + 1) * 64, :] if h < 2
                           else aT1[b][0:64, :])
                    {"v": nc.vector, "g": nc.gpsimd}[ENG["ttdiv"]].tensor_tensor(
                        dst, aTu[:], rb[:], OP.mult)

                # ---- final projection (bias via ones row) + store ----
                obuf = op_.tile([128, 6 * CO], F32, tag="obuf", name="obuf")
                otl = op_.tile([17, CO], F32, tag="otl", name="otl")
                for tb, (t0, tn) in enumerate(TBLK):
                    fp = psM.tile([128, 512], F32, tag="mm", name="mm")
                    nc.tensor.matmul(fp[0:tn, 0:CO], aT0[b][:, t0:t0 + tn],
                                     wpa0[:], start=True, stop=False)
                    nc.tensor.matmul(fp[0:tn, 0:CO], aT1[b][:, t0:t0 + tn],
                                     wpa1[:], start=False, stop=True)
                    dst = obuf[:, tb * CO:tb * CO + CO] if tb < 6 else otl[:]
                    copy_via("outcopy", dst[0:tn, :], fp[0:tn, 0:CO])
                nc.sync.dma_start(
                    out_d[b, 0:768, :].rearrange("(n p) c -> p n c", p=128),
                    obuf[:].rearrange("p (n c) -> p n c", n=6, c=CO))
                nc.sync.dma_start(out_d[b, 768:785, :], otl[:])

            # software pipeline: A(b) | attention T(b-1) | normalize N(b-2)
            for b in range(BPC):
                if b >= 2 and not NORM_AFTER_A:
                    emit_norm(b - 2)
                emit_A(b)
                if b >= 2 and NORM_AFTER_A:
                    emit_norm(b - 2)
                if b >= 1:
                    emit_attn(b - 1)
            emit_norm(BPC - 2)
            emit_attn(BPC - 1)
            emit_norm(BPC - 1)
    if not nc.is_finalized():
        nc.finalize()
    return nc


_NC_CACHE = None


def kernel(**inputs):
    global _NC_CACHE
    x = np.asarray(inputs["x"], dtype=np.float32)
    conv_w = np.asarray(inputs["conv_w"], dtype=np.float32)  # [3,C,1,3,3]
    bn_scale = np.asarray(inputs["bn_scale"], dtype=np.float32)
    bn_bias = np.asarray(inputs["bn_bias"], dtype=np.float32)
    bn_mean = np.asarray(inputs["bn_mean"], dtype=np.float32)
    bn_var = np.asarray(inputs["bn_var"], dtype=np.float32)
    w_qkv = np.asarray(inputs["w_qkv"], dtype=np.float32)  # [3,CO,C]
    w_proj = np.asarray(inputs["w_proj"], dtype=np.float32)  # [CO,CO]
    b_proj = np.asarray(inputs["b_proj"], dtype=np.float32)  # [CO]

    # fold BN into conv taps: y = conv(x, w)*s + (b - mu*s)
    s = bn_scale / np.sqrt(bn_var + BN_EPS)  # [3,C]
    wtap = (conv_w[:, :, 0, :, :].reshape(3, C, 9)
            * s[:, :, None]).astype(np.float32)  # [3,C,9]
    # [C, 27] with column i*9+tap
    wconv_h = np.ascontiguousarray(
        wtap.transpose(1, 0, 2).reshape(C, 27))
    bnt_h = np.ascontiguousarray(
        (bn_bias - bn_mean * s).T).astype(np.float32)  # [C,3]
    # diagonal tap matrices for the PE-side conv
    wdg0_h = np.zeros((27 * 128, 128), dtype=np.float16)
    wdg1_h = np.zeros((27 * 64, 64), dtype=np.float16)
    for i in range(3):
        for tap in range(9):
            k = i * 9 + tap
            d0 = wtap[i, 0:128, tap].astype(np.float16)
            d1 = wtap[i, 128:192, tap].astype(np.float16)
            wdg0_h[k * 128:(k + 1) * 128, :][np.arange(128), np.arange(128)] = d0
            wdg1_h[k * 64:(k + 1) * 64, :][np.arange(64), np.arange(64)] = d1
    # stacked (q,ch1)|(k,ch1) tap weights + bias for the fused DVE group
    wcs_h = np.zeros((128, 10), dtype=np.float32)
    wcs_h[0:64, 0:9] = wtap[0, 128:192, :]
    wcs_h[64:128, 0:9] = wtap[1, 128:192, :]
    wcs_h[0:64, 9] = bnt_h[128:192, 0]
    wcs_h[64:128, 9] = bnt_h[128:192, 1]
    wqkvT_h = np.ascontiguousarray(
        w_qkv.transpose(0, 2, 1)).astype(np.float16)  # [3,C,CO]
    wpa_h = np.concatenate(
        [w_proj.T, b_proj[None, :]], axis=0).astype(np.float16)

    if _NC_CACHE is None:
        _NC_CACHE = build_bass()
    nc = _NC_CACHE

    xs = x.reshape(NCORES, BPC, T, C)
    in_maps = [
        {"x": np.ascontiguousarray(xs[c]), "wqkvT": wqkvT_h,
         "wconv": wconv_h, "bnt": bnt_h, "wpa": wpa_h,
         "wdg0": wdg0_h, "wdg1": wdg1_h, "wcs": wcs_h}
        for c in range(NCORES)
    ]
    res = run_bass_kernel_spmd(nc, in_maps, list(range(NCORES)), **RUN_KWARGS)
    global LAST_RESULTS
    LAST_RESULTS = res
    out = np.concatenate([np.asarray(r["out"]) for r in res.results], axis=0)
    return out.reshape(B, T, CO).astype(np.float32)


RUN_KWARGS = {}
LAST_RESULTS = None
